# revision 24
# baseline (speedup 1.0000x reference)
"""BTV loss kernel for Trainium2 (8 NeuronCores, Bass/Tile).

reference: total = sum over 7x7 neighborhood shifts (k,l) != (0,0) of
           sqrt((x - roll(x,(k,l),axis=(2,3)))**2 + 1e-6).sum()
           out = 0.1 * total / x.size

Math:
  - circular-shift symmetry: shift (k,l) ~ (-k,-l); compute the 24
    half-space shifts {k>0, any l} u {k==0, l>0} and double.
  - sqrt(d^2 + 1e-6) ~= |d| (rel err ~3e-6); bf16 inputs add ~1e-5.

Engines (per 128-row block, 3 images per core):
  - DVE: custom CROSS op: one 2x instruction reads packed pairs of the
    base stream a and a shifted stream b (offset d) and accumulates
      |a_lo-b_lo| (shift d, even cols)   |a_hi-b_hi| (shift d, odd cols)
      |a_lo-b_hi| (shift d+1, even)      |a_hi-b_lo| (shift d-1, odd)
    i.e. 4 absdiff terms/cycle (2x the stock tensor_tensor rate). An
    11-instruction schedule covers shifts {k: l in -3..1} + (0,1..3)
    exactly (nd3/nd4 variants omit one cross term to avoid overlap).
  - PE+ACT: shifts (k,l) for k in 1..3, l in 2,3: PE computes
    diff = base - shift into PSUM via +I/-I matmuls; ACT does
    Abs + accum_out. 18 image-diffs/block in 9 pipelined groups of 2.
  - DMA: per block, tile_E (rows p, phase 0) and tile_O (rows p+j,
    j=0..3, columns shifted +1) are loaded straight from HBM; all
    odd column offsets come from tile_O so every DVE stream stays
    4-byte aligned with no on-chip repacking.

Distribution: pure data parallel over the 24 (b,c) images, 3 per core;
host sums the 8 per-core partials in f64.
"""

import dataclasses
import re
from operator import add as _py_add

import numpy as np

import concourse.bass as bass
import concourse.bacc as bacc_mod
import concourse.mybir as mybir
from concourse import dve_ops as _dvo
from concourse.dve_spec import AluOp as _DveAluOp
from concourse.dve_spec import Bin, Spec, Src0, Src1
from concourse.tile import TileContext
from concourse.bass_utils import run_bass_kernel_spmd

from concourse.dve_uop import (
    ENABLE,
    AluInp,
    AluOp as UAluOp,
    DelayInp,
    InpSel,
    OutPath,
    OutSel,
    Trigger,
    UopConfig,
)

B, C, H, W = 8, 3, 1024, 1024
NCORES = 8
IMGS = (B * C) // NCORES        # images per core = 3
BASE = 4                        # left col pad (even => 4B-aligned in bf16)
WP = W + BASE + 3 + 1           # 1032: [w-4..w-1][0..1023][0,1,2][pad]
RB = 128                        # rows per block (partition dim)
NBLK = H // RB                  # 8 row blocks per image
ROWS_BLK = RB + 3               # 131 rows stored per block (128 + 3 halo)
ROW = IMGS * WP                 # elements per stored row

WEIGHT = 0.1
F32 = mybir.dt.float32
BF16 = mybir.dt.bfloat16

# DVE schedule: (kind, variant, j=row phase, delta). Covers, per j>0,
# l in {-3..1} (ACT takes l=2,3), and for j=0 l in {1,2,3}:
#   full: F(d)+E(d+1)+O(d-1);  nd3: F(d)+O(d-1);  nd4: F(d)+E(d+1)
K0_SCHED = [("full", 1), ("nd3", 3)]     # (variant, delta), W-streams
TRIO_SCHED = [("nd4", -3), ("full", -1), ("full", 1)]  # j-merged full-row
# PE/ACT shifts: (j, l) pairs
ACT_SHIFTS = [(j, l) for j in (1, 2, 3) for l in (2, 3)]
ACT_GRP = 2  # image-diffs per PSUM tile / ACT instruction (4 PSUM banks)


def _mk_cross_uop(kind: str, use_d3: bool, use_d4: bool):
    """2x CROSS uop. kind: "seed" (acc <- sum on elem 0) | "steady".
    blocks: 0:d1  1:d4  2:d3  3:d2  4..6:sum tree  7:acc
    Omitted terms compute ABSDIFF(x, x) = 0 instead (same structure).
    """
    u = UopConfig()
    u.enable_input(InpSel.SRC_0, 0)      # a_lo -> ALU lane
    u.enable_input(InpSel.SRC_1, 1)      # b_lo -> delay lane 0
    u.enable_input(InpSel.SRC_0_HI, 2)   # a_hi -> delay lane 1
    u.enable_input(InpSel.SRC_1_HI, 3)   # b_hi -> delay lane 2
    u.accum_enabled = ENABLE
    dp = u.datapath_config
    dp[0].enable_alu(UAluOp.ABSOLUTE_DIFF, AluInp.PREV_ALU_OUT, AluInp.PREV_DELAY_0)
    dp[0].enable_delay_from_src(DelayInp.PREV_ALU_OUT, 3)
    dp[0].pass_through_delay(0, 1, 2)
    dp[1].enable_alu(
        UAluOp.ABSOLUTE_DIFF,
        AluInp.PREV_DELAY_1,
        AluInp.PREV_DELAY_0 if use_d4 else AluInp.PREV_DELAY_1,
    )
    dp[1].enable_delay_from_src(DelayInp.PREV_ALU_OUT, 0)
    dp[1].pass_through_delay(1, 2, 3)
    dp[2].enable_alu(
        UAluOp.ABSOLUTE_DIFF,
        AluInp.PREV_DELAY_3,
        AluInp.PREV_DELAY_2 if use_d3 else AluInp.PREV_DELAY_3,
    )
    dp[2].enable_delay_from_src(DelayInp.PREV_ALU_OUT, 3)
    dp[2].pass_through_delay(0, 1, 2)
    dp[3].enable_alu(UAluOp.ABSOLUTE_DIFF, AluInp.PREV_DELAY_1, AluInp.PREV_DELAY_2)
    dp[3].enable_delay_from_src(DelayInp.PREV_ALU_OUT, 1)
    dp[3].pass_through_delay(0, 3)
    dp[4].enable_alu(UAluOp.ADD, AluInp.PREV_ALU_OUT, AluInp.PREV_DELAY_1)
    dp[4].pass_through_delay(0, 3)
    dp[5].enable_alu(UAluOp.ADD, AluInp.PREV_ALU_OUT, AluInp.PREV_DELAY_0)
    dp[5].pass_through_delay(3)
    dp[6].enable_alu(UAluOp.ADD, AluInp.PREV_ALU_OUT, AluInp.PREV_DELAY_3)
    if kind == "seed":
        dp[7].enable_alu(UAluOp.BYPASS, AluInp.PREV_ALU_OUT, AluInp.PREV_ALU_OUT)
    else:
        dp[7].enable_alu(UAluOp.ADD, AluInp.CURR_ALU_OUT, AluInp.PREV_ALU_OUT)
    dp[7].alu_out_a_enable = ENABLE
    u.require_inp0 = ENABLE
    u.require_inp1 = ENABLE
    u.enable_output(OutSel.ALU_OUT, OutPath.WR0_LO)
    u.enable_output(OutSel.ALU_OUT, OutPath.WR0_HI)
    if kind == "seed":
        u.trigger = (Trigger.COUNT, Trigger.SRC_TENSOR_DONE, Trigger.NONE)
        u.next_uop = (1, 0, 0)
        u.repeat_count = 1
    else:
        u.trigger = (Trigger.SRC_TENSOR_DONE, Trigger.NONE, Trigger.NONE)
        u.next_uop = (0, 0, 0)
    return u


def _mk_poison_1x():
    """1x fallback: acc <- +inf so any non-2x execution is caught."""
    u = UopConfig()
    u.enable_input(InpSel.SRC_0, 0)
    u.enable_input(InpSel.POS_INF, 1)
    dp = u.datapath_config
    for b in range(7):
        dp[b].enable_alu(UAluOp.BYPASS, AluInp.PREV_ALU_OUT, AluInp.PREV_ALU_OUT)
        dp[b].pass_through_delay(0)
    dp[7].enable_alu(UAluOp.BYPASS, AluInp.PREV_DELAY_0, AluInp.PREV_DELAY_0)
    dp[7].alu_out_a_enable = ENABLE
    u.accum_enabled = ENABLE
    u.require_inp0 = ENABLE
    u.require_inp1 = ENABLE
    u.enable_output(OutSel.ALU_OUT, OutPath.WR0_LO)
    u.trigger = (Trigger.SRC_TENSOR_DONE, Trigger.NONE, Trigger.NONE)
    u.next_uop = (0, 0, 0)
    return u


def _mk_read_uop():
    """Route blk7's accumulator flop to the output (1-element stream)."""
    u = UopConfig()
    u.enable_input(InpSel.SRC_0, 0)
    dp = u.datapath_config
    for b in range(7):
        dp[b].enable_alu(UAluOp.BYPASS, AluInp.PREV_ALU_OUT, AluInp.PREV_ALU_OUT)
    dp[7].enable_alu(UAluOp.BYPASS, AluInp.CURR_ALU_OUT, AluInp.CURR_ALU_OUT)
    u.require_inp0 = ENABLE
    u.enable_output(OutSel.ALU_OUT, OutPath.WR0_LO)
    u.trigger = (Trigger.SRC_TENSOR_DONE, Trigger.NONE, Trigger.NONE)
    u.next_uop = (0, 0, 0)
    return u


class _HandDveOp(_dvo.DveOp):
    BUILDERS = {}  # name -> (build_1x_list, build_2x_list_or_None, rd1_en)

    def compile(self, ver):
        from concourse.dve_uop import DveOpSpec

        key = (self.name, ver)
        if (r := _dvo._COMPILE_CACHE.get(key)) is not None:
            return r
        b1, b2, rd1 = self.BUILDERS[self.name]
        result = DveOpSpec(
            name=self.name,
            opcode=_dvo.get_dve_sub_opcode(self.name),
            uops=b1(),
            uops_2x=(b2() if b2 is not None else None),
            rd1_en=rd1,
        )
        got = result.sha(ver)
        if self.uops_sha.get(ver) != got:
            raise ValueError(f"sha drift ({ver}: {got} != pinned)")
        _dvo._COMPILE_CACHE[key] = result
        return result


def _register(name, spec, build_1x, build_2x, rd1_en):
    _HandDveOp.BUILDERS[name] = (build_1x, build_2x, rd1_en)
    op = _HandDveOp(name, spec, subdim=False, uops_sha={})
    _dvo._SUB_OPCODE_FOR_NAME[name] = _dvo._CUSTOM_DVE_ROW_BASE + len(_dvo.OPS)
    shas = {}
    for ver in ("v3", "v4"):
        try:
            op.compile(ver)
            shas[ver] = op.uops_sha.get(ver)
        except ValueError as e:
            m = re.search(r"([0-9a-f]{16})", str(e))
            if not m:
                raise
            shas[ver] = m.group(1)
    op = dataclasses.replace(op, uops_sha=shas)
    _dvo.OPS.append(op)
    _dvo.CUSTOM_DVE_SPECS[name] = spec
    return op


_OPS = None


def _get_ops():
    """dict: (kind, variant) -> op, plus 'read'."""
    global _OPS
    if _OPS is not None:
        return _OPS
    have = {op.name: op for op in _dvo.OPS}
    names = {
        ("seed", "full"): "XR_SEED_F_ANT",
        ("seed", "nd3"): "XR_SEED_ND3_ANT",
        ("seed", "nd4"): "XR_SEED_ND4_ANT",
        ("cont", "full"): "XR_CONT_F_ANT",
        ("cont", "nd3"): "XR_CONT_ND3_ANT",
        ("cont", "nd4"): "XR_CONT_ND4_ANT",
    }
    if names[("seed", "full")] in have:
        _OPS = {k: have[n] for k, n in names.items()}
        _OPS["read"] = have["XR_READ_ANT"]
        return _OPS

    def _ref(in0, in1, s0, s1, imm2):
        a = in0.astype(np.float32)
        b = in1.astype(np.float32)
        P = a.shape[0]
        out = np.abs(a.reshape(P, -1) - b.reshape(P, -1))
        return out.reshape(in0.shape), out.reshape(P, -1).sum(-1, keepdims=True)

    spec_acc = Spec(
        body=Bin(_DveAluOp.ABSOLUTE_DIFF, Src0, Src1),
        accum=_py_add,
        reference=_ref,
    )
    spec_read = Spec(
        body=Src0,
        reference=lambda in0, in1, s0, s1, imm2: in0.astype(np.float32),
    )
    _OPS = {}
    for (kind, var), name in names.items():
        d3, d4 = var != "nd3", var != "nd4"
        _OPS[(kind, var)] = _register(
            name,
            spec_acc,
            lambda: [_mk_poison_1x(), _mk_poison_1x()],
            lambda kind=kind, d3=d3, d4=d4: [
                _mk_cross_uop(kind, d3, d4),
                _mk_cross_uop("steady", d3, d4),
            ],
            True,
        )
    _OPS["read"] = _register(
        "XR_READ_ANT", spec_read, lambda: [_mk_read_uop()], None, False
    )
    return _OPS


NGRP = 7  # ceil(27 512-wide slots / 4) ACT groups per block
STAGE_COLS = 1 + NBLK * NGRP


def _build_nc():
    ops = _get_ops()
    nc = bacc_mod.Bacc("TRN2", target_bir_lowering=False)
    # host layout: flat; stored[r, q, i, c] = block r, row 128r+q, img i,
    # col c (with BASE left pad / 4 right cols); +8 elements tail pad so
    # the tile_O DMA (+1 element offset) stays in bounds.
    X = nc.dram_tensor(
        "x", [NBLK * ROWS_BLK * ROW + 8], BF16, kind="ExternalInput"
    )
    WI = nc.dram_tensor("wi", [128, 128], BF16, kind="ExternalInput")
    WNI = nc.dram_tensor("wni", [128, 128], BF16, kind="ExternalInput")
    OUT = nc.dram_tensor("out", [128, STAGE_COLS], F32, kind="ExternalOutput")

    with TileContext(nc) as tc:
        with (
            tc.tile_pool(name="te", bufs=2) as te_pool,
            tc.tile_pool(name="to01", bufs=2) as to01_pool,
            tc.tile_pool(name="to23", bufs=2) as to23_pool,
            tc.tile_pool(name="sc", bufs=1) as sc_pool,
            tc.tile_pool(name="acc", bufs=1) as acc_pool,
            tc.psum_pool(name="ps", bufs=2) as ps_pool,
        ):
            stage = acc_pool.tile([128, STAGE_COLS], F32)
            scratch = sc_pool.tile([128, 3 * ROW], BF16)
            ascr = acc_pool.tile([128, ACT_GRP * W], BF16)
            wi = acc_pool.tile([128, 128], BF16)
            wni = acc_pool.tile([128, 128], BF16)
            nc.sync.dma_start(out=wi[:], in_=WI[:])
            nc.sync.dma_start(out=wni[:], in_=WNI[:])
            # pre-load the ACT Abs table before any DVE critical section
            # (the lazy table-load DMA deadlocks against critical branches)
            nc.scalar.activation(
                out=ascr[:, 0:2],
                in_=wi[:, 0:2],
                func=mybir.ActivationFunctionType.Abs,
            )
            for r in range(NBLK):
                te = te_pool.tile([128, IMGS, WP], BF16, tag="te")
                to0 = to01_pool.tile([128, IMGS, WP], BF16, tag="to0")
                t123 = to23_pool.tile([128, 3 * ROW + 8], BF16, tag="t123")
                # Blocks 0-1 FIFO on the sync ring (full HBM rate for the
                # pipeline fill); later blocks' big prefetch via GPSIMD's
                # SWDGE so the busy ACT engine issues no DMA.
                eng_b = nc.scalar if r <= 1 else nc.gpsimd
                nc.sync.dma_start(
                    out=te[:],
                    in_=bass.AP(X, r * ROWS_BLK * ROW, [[ROW, 128], [1, ROW]]),
                )
                nc.sync.dma_start(
                    out=to0[:],
                    in_=bass.AP(
                        X, r * ROWS_BLK * ROW + 1, [[ROW, 128], [1, ROW]]
                    ),
                )
                eng_b.dma_start(
                    out=t123[:],
                    in_=bass.AP(
                        X,
                        (r * ROWS_BLK + 1) * ROW - 3,
                        [[ROW, 128], [1, 3 * ROW + 8]],
                    ),
                )
                base = te[:, :, BASE : BASE + W]

                # --- PE + ACT: l=3 full-width; l=2 odd columns only
                # (the delta=1 full trio covers even columns of l=2 on
                # DVE). 512-wide slots, 4 per PSUM group.
                MMW = 512
                slots = []  # (base_rhs, shift_rhs) pairs, each 512 wide
                for (j, l) in ACT_SHIFTS:
                    for i in range(IMGS):
                        cb = (j - 1) * ROW + 3 + i * WP + BASE + l
                        if l == 3:
                            for c0 in range(0, W, MMW):
                                slots.append((
                                    te[:, i, BASE + c0 : BASE + c0 + MMW],
                                    t123[:, cb + c0 : cb + c0 + MMW],
                                ))
                        else:
                            bo = te[:, i, BASE + 1 : BASE + 1 + W]
                            bo = bo.rearrange("p (c t) -> p c t", t=2)[:, :, 0]
                            so = t123[:, cb + 1 : cb + 1 + W]
                            so = so.rearrange("p (c t) -> p c t", t=2)[:, :, 0]
                            slots.append((bo, so))
                for g in range(0, len(slots), 4):
                    grp = slots[g : g + 4]
                    psum = ps_pool.tile([128, 4 * MMW], F32, tag="ps")
                    for m, (brhs, srhs) in enumerate(grp):
                        nc.tensor.matmul(
                            out=psum[:, m * MMW : (m + 1) * MMW],
                            lhsT=wi[:],
                            rhs=brhs,
                            start=True,
                            stop=False,
                        )
                    for m, (brhs, srhs) in enumerate(grp):
                        nc.tensor.matmul(
                            out=psum[:, m * MMW : (m + 1) * MMW],
                            lhsT=wni[:],
                            rhs=srhs,
                            start=False,
                            stop=True,
                        )
                    col = 1 + r * NGRP + g // 4
                    nc.scalar.activation(
                        out=psum[:, 0 : len(grp) * MMW],
                        in_=psum[:, 0 : len(grp) * MMW],
                        func=mybir.ActivationFunctionType.Abs,
                        accum_out=stage[:, col : col + 1],
                    )

                # --- DVE chain: 2 k=0 ops + 3 j-merged trios + (one
                # final read). All ops write the shared scratch tile so
                # WAW deps keep the chain contiguous under the scheduler.
                for n, (var, delta) in enumerate(K0_SCHED):
                    kind = "seed" if (n == 0 and r == 0) else "cont"
                    c0 = BASE + delta - 1
                    nc.vector._custom_dve(
                        ops[(kind, var)],
                        out=scratch[:, 0 : IMGS * W],
                        in0=base,
                        in1=to0[:, :, c0 : c0 + W],
                    ).ins.perf_max = 1
                in0b = te[:].rearrange("p a b -> p (a b)")
                in0b = in0b.rearrange("p (x c) -> p x c", x=1)
                in0b = in0b.broadcast_to((128, 3, ROW))
                for var, delta in TRIO_SCHED:
                    m0 = delta + 3
                    in1b = t123[:, m0 : m0 + 3 * ROW].rearrange(
                        "p (j c) -> p j c", j=3
                    )
                    nc.vector._custom_dve(
                        ops[("cont", var)],
                        out=scratch[:],
                        in0=in0b,
                        in1=in1b,
                    ).ins.perf_max = 1
            nc.vector._custom_dve(
                ops["read"],
                out=stage[:, 0:1],
                in0=scratch[:, 0:1],
            )
            nc.sync.dma_start(out=OUT[:], in_=stage[:])
    return nc


_NC = None


def _get_nc():
    global _NC
    if _NC is None:
        _NC = _build_nc()
        if not _NC.is_finalized():
            _NC.finalize()
    return _NC


def _prep_shards(x: np.ndarray) -> list[dict[str, np.ndarray]]:
    """bf16-cast, circular pad, blockify into the flat per-core layout."""
    imgs = np.ascontiguousarray(x.reshape(B * C, H, W), dtype=np.float32)

    def to_bf16(a32):
        b = a32.view(np.uint32)
        return ((b + 0x7FFF + ((b >> 16) & 1)) >> 16).astype(np.uint16)

    imgs_b = to_bf16(imgs)  # (24, H, W) uint16 view of bf16
    HPAD = H + 3
    even = np.zeros((B * C, HPAD, WP), dtype=np.uint16)
    even[:, :H, BASE : BASE + W] = imgs_b
    even[:, :H, :BASE] = imgs_b[:, :, W - BASE :]
    even[:, :H, BASE + W : BASE + W + 3] = imgs_b[:, :, :3]
    even[:, H:, :] = even[:, :3, :]

    I = np.eye(128, dtype=np.float32)
    wi = to_bf16(I)
    wni = to_bf16(-I)

    shards = even.reshape(NCORES, IMGS, HPAD, WP)
    out = []
    pcorr = []
    for n in range(NCORES):
        t = shards[n].transpose(1, 0, 2)  # (HPAD, IMGS, WP)
        blk = np.empty((NBLK, ROWS_BLK, IMGS, WP), dtype=np.uint16)
        for r in range(NBLK):
            blk[r] = t[r * RB : r * RB + ROWS_BLK]
        flat = np.concatenate([blk.reshape(-1), np.zeros(8, np.uint16)])
        out.append({"x": flat, "wi": wi, "wni": wni})
        # Exact correction for the merged-trio ops' pad-column terms:
        # streams span full WP rows, so a-columns [0,BASE) u [BASE+W,WP)
        # contribute deterministic extra |a-b| terms (b read at flat
        # offset +j*ROW+delta, exactly as the device tile is laid out).
        af = (flat.astype(np.uint32) << 16).view(np.float32).astype(np.float64)
        rowbase = (
            (ROWS_BLK * np.arange(NBLK)[:, None] + np.arange(128)[None, :])
            * ROW
        )
        P = 0.0
        for j in (1, 2, 3):
            for var, dlt in TRIO_SCHED:
                for i in range(IMGS):
                    for c in (0, 2, BASE + W, BASE + W + 2):
                        ai = rowbase + i * WP + c
                        bi = ai + j * ROW + dlt
                        alo, ahi = af[ai], af[ai + 1]
                        blo, bhi = af[bi], af[bi + 1]
                        t = np.abs(alo - blo) + np.abs(ahi - bhi)
                        if var != "nd3":
                            t += np.abs(alo - bhi)
                        if var != "nd4":
                            t += np.abs(ahi - blo)
                        P += t.sum()
        pcorr.append(P)
    return out, pcorr


def _run(x: np.ndarray, trace: bool = False):
    import ml_dtypes

    nc = _get_nc()
    in_maps, pcorr = _prep_shards(x)
    in_maps = [
        {k: v.view(ml_dtypes.bfloat16) for k, v in m.items()} for m in in_maps
    ]
    res = run_bass_kernel_spmd(
        nc, in_maps, core_ids=list(range(NCORES)), trace=trace
    )
    total = 0.0
    for r, pc in zip(res.results, pcorr):
        total += r["out"].astype(np.float64).sum() - pc
    val = WEIGHT * 2.0 * total / float(B * C * H * W)
    return np.float32(val), res


def kernel(x: np.ndarray) -> np.ndarray:
    x = np.asarray(x, dtype=np.float32)
    val, _ = _run(x, trace=False)
    return val


# revision 25
# speedup vs baseline: 1.0569x; 1.0569x over previous
"""BTV loss kernel for Trainium2 (8 NeuronCores, Bass/Tile).

reference: total = sum over 7x7 neighborhood shifts (k,l) != (0,0) of
           sqrt((x - roll(x,(k,l),axis=(2,3)))**2 + 1e-6).sum()
           out = 0.1 * total / x.size

Math:
  - circular-shift symmetry: shift (k,l) ~ (-k,-l); compute the 24
    half-space shifts {k>0, any l} u {k==0, l>0} and double.
  - sqrt(d^2 + 1e-6) ~= |d| (rel err ~3e-6); bf16 inputs add ~1e-5.

Engines (per 128-row block, 3 images per core):
  - DVE: custom CROSS op: one 2x instruction reads packed pairs of the
    base stream a and a shifted stream b (offset d) and accumulates
      |a_lo-b_lo| (shift d, even cols)   |a_hi-b_hi| (shift d, odd cols)
      |a_lo-b_hi| (shift d+1, even)      |a_hi-b_lo| (shift d-1, odd)
    i.e. 4 absdiff terms/cycle (2x the stock tensor_tensor rate). An
    11-instruction schedule covers shifts {k: l in -3..1} + (0,1..3)
    exactly (nd3/nd4 variants omit one cross term to avoid overlap).
  - PE+ACT: shifts (k,l) for k in 1..3, l in 2,3: PE computes
    diff = base - shift into PSUM via +I/-I matmuls; ACT does
    Abs + accum_out. 18 image-diffs/block in 9 pipelined groups of 2.
  - DMA: per block, tile_E (rows p, phase 0) and tile_O (rows p+j,
    j=0..3, columns shifted +1) are loaded straight from HBM; all
    odd column offsets come from tile_O so every DVE stream stays
    4-byte aligned with no on-chip repacking.

Distribution: pure data parallel over the 24 (b,c) images, 3 per core;
host sums the 8 per-core partials in f64.
"""

import dataclasses
import re
from operator import add as _py_add

import numpy as np

import concourse.bass as bass
import concourse.bacc as bacc_mod
import concourse.mybir as mybir
from concourse import dve_ops as _dvo
from concourse.dve_spec import AluOp as _DveAluOp
from concourse.dve_spec import Bin, Spec, Src0, Src1
from concourse.tile import TileContext
from concourse.bass_utils import run_bass_kernel_spmd

from concourse.dve_uop import (
    ENABLE,
    AluInp,
    AluOp as UAluOp,
    DelayInp,
    InpSel,
    OutPath,
    OutSel,
    Trigger,
    UopConfig,
)

B, C, H, W = 8, 3, 1024, 1024
NCORES = 8
IMGS = (B * C) // NCORES        # images per core = 3
BASE = 4                        # left col pad (even => 4B-aligned in bf16)
WP = W + BASE + 3 + 1           # 1032: [w-4..w-1][0..1023][0,1,2][pad]
RB = 128                        # rows per block (partition dim)
NBLK = H // RB                  # 8 row blocks per image
ROWS_BLK = RB + 3               # 131 rows stored per block (128 + 3 halo)
ROW = IMGS * WP                 # elements per stored row

WEIGHT = 0.1
F32 = mybir.dt.float32
BF16 = mybir.dt.bfloat16

# DVE schedule: (kind, variant, j=row phase, delta). Covers, per j>0,
# l in {-3..1} (ACT takes l=2,3), and for j=0 l in {1,2,3}:
#   full: F(d)+E(d+1)+O(d-1);  nd3: F(d)+O(d-1);  nd4: F(d)+E(d+1)
K0_SCHED = [("full", 1), ("nd3", 3)]     # (variant, delta), W-streams
TRIO_SCHED = [("nd4", -3), ("full", -1), ("full", 1)]  # j-merged full-row
# PE/ACT shifts: (j, l) pairs
ACT_SHIFTS = [(j, l) for j in (1, 2, 3) for l in (2, 3)]
ACT_GRP = 2  # image-diffs per PSUM tile / ACT instruction (4 PSUM banks)


def _mk_cross_uop(kind: str, use_d3: bool, use_d4: bool):
    """2x CROSS uop. kind: "seed" (acc <- sum on elem 0) | "steady".
    blocks: 0:d1  1:d4  2:d3  3:d2  4..6:sum tree  7:acc
    Omitted terms compute ABSDIFF(x, x) = 0 instead (same structure).
    """
    u = UopConfig()
    u.enable_input(InpSel.SRC_0, 0)      # a_lo -> ALU lane
    u.enable_input(InpSel.SRC_1, 1)      # b_lo -> delay lane 0
    u.enable_input(InpSel.SRC_0_HI, 2)   # a_hi -> delay lane 1
    u.enable_input(InpSel.SRC_1_HI, 3)   # b_hi -> delay lane 2
    u.accum_enabled = ENABLE
    dp = u.datapath_config
    dp[0].enable_alu(UAluOp.ABSOLUTE_DIFF, AluInp.PREV_ALU_OUT, AluInp.PREV_DELAY_0)
    dp[0].enable_delay_from_src(DelayInp.PREV_ALU_OUT, 3)
    dp[0].pass_through_delay(0, 1, 2)
    dp[1].enable_alu(
        UAluOp.ABSOLUTE_DIFF,
        AluInp.PREV_DELAY_1,
        AluInp.PREV_DELAY_0 if use_d4 else AluInp.PREV_DELAY_1,
    )
    dp[1].enable_delay_from_src(DelayInp.PREV_ALU_OUT, 0)
    dp[1].pass_through_delay(1, 2, 3)
    dp[2].enable_alu(
        UAluOp.ABSOLUTE_DIFF,
        AluInp.PREV_DELAY_3,
        AluInp.PREV_DELAY_2 if use_d3 else AluInp.PREV_DELAY_3,
    )
    dp[2].enable_delay_from_src(DelayInp.PREV_ALU_OUT, 3)
    dp[2].pass_through_delay(0, 1, 2)
    dp[3].enable_alu(UAluOp.ABSOLUTE_DIFF, AluInp.PREV_DELAY_1, AluInp.PREV_DELAY_2)
    dp[3].enable_delay_from_src(DelayInp.PREV_ALU_OUT, 1)
    dp[3].pass_through_delay(0, 3)
    dp[4].enable_alu(UAluOp.ADD, AluInp.PREV_ALU_OUT, AluInp.PREV_DELAY_1)
    dp[4].pass_through_delay(0, 3)
    dp[5].enable_alu(UAluOp.ADD, AluInp.PREV_ALU_OUT, AluInp.PREV_DELAY_0)
    dp[5].pass_through_delay(3)
    dp[6].enable_alu(UAluOp.ADD, AluInp.PREV_ALU_OUT, AluInp.PREV_DELAY_3)
    if kind == "seed":
        dp[7].enable_alu(UAluOp.BYPASS, AluInp.PREV_ALU_OUT, AluInp.PREV_ALU_OUT)
    else:
        dp[7].enable_alu(UAluOp.ADD, AluInp.CURR_ALU_OUT, AluInp.PREV_ALU_OUT)
    dp[7].alu_out_a_enable = ENABLE
    u.require_inp0 = ENABLE
    u.require_inp1 = ENABLE
    u.enable_output(OutSel.ALU_OUT, OutPath.WR0_LO)
    u.enable_output(OutSel.ALU_OUT, OutPath.WR0_HI)
    if kind == "seed":
        u.trigger = (Trigger.COUNT, Trigger.SRC_TENSOR_DONE, Trigger.NONE)
        u.next_uop = (1, 0, 0)
        u.repeat_count = 1
    else:
        u.trigger = (Trigger.SRC_TENSOR_DONE, Trigger.NONE, Trigger.NONE)
        u.next_uop = (0, 0, 0)
    return u


def _mk_poison_1x():
    """1x fallback: acc <- +inf so any non-2x execution is caught."""
    u = UopConfig()
    u.enable_input(InpSel.SRC_0, 0)
    u.enable_input(InpSel.POS_INF, 1)
    dp = u.datapath_config
    for b in range(7):
        dp[b].enable_alu(UAluOp.BYPASS, AluInp.PREV_ALU_OUT, AluInp.PREV_ALU_OUT)
        dp[b].pass_through_delay(0)
    dp[7].enable_alu(UAluOp.BYPASS, AluInp.PREV_DELAY_0, AluInp.PREV_DELAY_0)
    dp[7].alu_out_a_enable = ENABLE
    u.accum_enabled = ENABLE
    u.require_inp0 = ENABLE
    u.require_inp1 = ENABLE
    u.enable_output(OutSel.ALU_OUT, OutPath.WR0_LO)
    u.trigger = (Trigger.SRC_TENSOR_DONE, Trigger.NONE, Trigger.NONE)
    u.next_uop = (0, 0, 0)
    return u


def _mk_read_uop():
    """Route blk7's accumulator flop to the output (1-element stream)."""
    u = UopConfig()
    u.enable_input(InpSel.SRC_0, 0)
    dp = u.datapath_config
    for b in range(7):
        dp[b].enable_alu(UAluOp.BYPASS, AluInp.PREV_ALU_OUT, AluInp.PREV_ALU_OUT)
    dp[7].enable_alu(UAluOp.BYPASS, AluInp.CURR_ALU_OUT, AluInp.CURR_ALU_OUT)
    u.require_inp0 = ENABLE
    u.enable_output(OutSel.ALU_OUT, OutPath.WR0_LO)
    u.trigger = (Trigger.SRC_TENSOR_DONE, Trigger.NONE, Trigger.NONE)
    u.next_uop = (0, 0, 0)
    return u


class _HandDveOp(_dvo.DveOp):
    BUILDERS = {}  # name -> (build_1x_list, build_2x_list_or_None, rd1_en)

    def compile(self, ver):
        from concourse.dve_uop import DveOpSpec

        key = (self.name, ver)
        if (r := _dvo._COMPILE_CACHE.get(key)) is not None:
            return r
        b1, b2, rd1 = self.BUILDERS[self.name]
        result = DveOpSpec(
            name=self.name,
            opcode=_dvo.get_dve_sub_opcode(self.name),
            uops=b1(),
            uops_2x=(b2() if b2 is not None else None),
            rd1_en=rd1,
        )
        got = result.sha(ver)
        if self.uops_sha.get(ver) != got:
            raise ValueError(f"sha drift ({ver}: {got} != pinned)")
        _dvo._COMPILE_CACHE[key] = result
        return result


def _register(name, spec, build_1x, build_2x, rd1_en):
    _HandDveOp.BUILDERS[name] = (build_1x, build_2x, rd1_en)
    op = _HandDveOp(name, spec, subdim=False, uops_sha={})
    _dvo._SUB_OPCODE_FOR_NAME[name] = _dvo._CUSTOM_DVE_ROW_BASE + len(_dvo.OPS)
    shas = {}
    for ver in ("v3", "v4"):
        try:
            op.compile(ver)
            shas[ver] = op.uops_sha.get(ver)
        except ValueError as e:
            m = re.search(r"([0-9a-f]{16})", str(e))
            if not m:
                raise
            shas[ver] = m.group(1)
    op = dataclasses.replace(op, uops_sha=shas)
    _dvo.OPS.append(op)
    _dvo.CUSTOM_DVE_SPECS[name] = spec
    return op


_OPS = None


def _get_ops():
    """dict: (kind, variant) -> op, plus 'read'."""
    global _OPS
    if _OPS is not None:
        return _OPS
    have = {op.name: op for op in _dvo.OPS}
    names = {
        ("seed", "full"): "XR_SEED_F_ANT",
        ("seed", "nd3"): "XR_SEED_ND3_ANT",
        ("seed", "nd4"): "XR_SEED_ND4_ANT",
        ("cont", "full"): "XR_CONT_F_ANT",
        ("cont", "nd3"): "XR_CONT_ND3_ANT",
        ("cont", "nd4"): "XR_CONT_ND4_ANT",
    }
    if names[("seed", "full")] in have:
        _OPS = {k: have[n] for k, n in names.items()}
        _OPS["read"] = have["XR_READ_ANT"]
        return _OPS

    def _ref(in0, in1, s0, s1, imm2):
        a = in0.astype(np.float32)
        b = in1.astype(np.float32)
        P = a.shape[0]
        out = np.abs(a.reshape(P, -1) - b.reshape(P, -1))
        return out.reshape(in0.shape), out.reshape(P, -1).sum(-1, keepdims=True)

    spec_acc = Spec(
        body=Bin(_DveAluOp.ABSOLUTE_DIFF, Src0, Src1),
        accum=_py_add,
        reference=_ref,
    )
    spec_read = Spec(
        body=Src0,
        reference=lambda in0, in1, s0, s1, imm2: in0.astype(np.float32),
    )
    _OPS = {}
    for (kind, var), name in names.items():
        d3, d4 = var != "nd3", var != "nd4"
        _OPS[(kind, var)] = _register(
            name,
            spec_acc,
            lambda: [_mk_poison_1x(), _mk_poison_1x()],
            lambda kind=kind, d3=d3, d4=d4: [
                _mk_cross_uop(kind, d3, d4),
                _mk_cross_uop("steady", d3, d4),
            ],
            True,
        )
    _OPS["read"] = _register(
        "XR_READ_ANT", spec_read, lambda: [_mk_read_uop()], None, False
    )
    return _OPS


NGRP = 7  # ceil(27 512-wide slots / 4) ACT groups per block
STAGE_COLS = 1 + NBLK * NGRP


def _build_nc():
    ops = _get_ops()
    nc = bacc_mod.Bacc("TRN2", target_bir_lowering=False)
    # host layout: flat; stored[r, q, i, c] = block r, row 128r+q, img i,
    # col c (with BASE left pad / 4 right cols); +8 elements tail pad so
    # the tile_O DMA (+1 element offset) stays in bounds.
    X = nc.dram_tensor(
        "x", [NBLK * ROWS_BLK * ROW + 8], BF16, kind="ExternalInput"
    )
    WI = nc.dram_tensor("wi", [128, 128], BF16, kind="ExternalInput")
    WNI = nc.dram_tensor("wni", [128, 128], BF16, kind="ExternalInput")
    OUT = nc.dram_tensor("out", [128, STAGE_COLS], F32, kind="ExternalOutput")

    with TileContext(nc) as tc:
        with (
            tc.tile_pool(name="te", bufs=2) as te_pool,
            tc.tile_pool(name="to01", bufs=2) as to01_pool,
            tc.tile_pool(name="to23", bufs=2) as to23_pool,
            tc.tile_pool(name="sc", bufs=1) as sc_pool,
            tc.tile_pool(name="acc", bufs=1) as acc_pool,
            tc.psum_pool(name="ps", bufs=2) as ps_pool,
        ):
            stage = acc_pool.tile([128, STAGE_COLS], F32)
            scratch = sc_pool.tile([128, 3 * ROW], BF16)
            ascr = acc_pool.tile([128, ACT_GRP * W], BF16)
            wi = acc_pool.tile([128, 128], BF16)
            wni = acc_pool.tile([128, 128], BF16)
            nc.sync.dma_start(out=wi[:], in_=WI[:])
            nc.sync.dma_start(out=wni[:], in_=WNI[:])
            # pre-load the ACT Abs table before any DVE critical section
            # (the lazy table-load DMA deadlocks against critical branches)
            nc.scalar.activation(
                out=ascr[:, 0:2],
                in_=wi[:, 0:2],
                func=mybir.ActivationFunctionType.Abs,
            )
            for r in range(NBLK):
                te = te_pool.tile([128, IMGS, WP], BF16, tag="te")
                to0 = to01_pool.tile([128, IMGS, WP], BF16, tag="to0")
                t123 = to23_pool.tile([128, 3 * ROW + 8], BF16, tag="t123")
                # Blocks 0-1 FIFO on the sync ring (full HBM rate for the
                # pipeline fill); later blocks' big prefetch via GPSIMD's
                # SWDGE so the busy ACT engine issues no DMA.
                eng_b = nc.sync if r <= 1 else nc.gpsimd
                nc.sync.dma_start(
                    out=te[:],
                    in_=bass.AP(X, r * ROWS_BLK * ROW, [[ROW, 128], [1, ROW]]),
                )
                nc.sync.dma_start(
                    out=to0[:],
                    in_=bass.AP(
                        X, r * ROWS_BLK * ROW + 1, [[ROW, 128], [1, ROW]]
                    ),
                )
                eng_b.dma_start(
                    out=t123[:],
                    in_=bass.AP(
                        X,
                        (r * ROWS_BLK + 1) * ROW - 3,
                        [[ROW, 128], [1, 3 * ROW + 8]],
                    ),
                )
                base = te[:, :, BASE : BASE + W]

                # --- PE + ACT: l=3 full-width; l=2 odd columns only
                # (the delta=1 full trio covers even columns of l=2 on
                # DVE). 512-wide slots, 4 per PSUM group.
                MMW = 512
                slots = []  # (base_rhs, shift_rhs) pairs, each 512 wide
                for (j, l) in ACT_SHIFTS:
                    for i in range(IMGS):
                        cb = (j - 1) * ROW + 3 + i * WP + BASE + l
                        if l == 3:
                            for c0 in range(0, W, MMW):
                                slots.append((
                                    te[:, i, BASE + c0 : BASE + c0 + MMW],
                                    t123[:, cb + c0 : cb + c0 + MMW],
                                ))
                        else:
                            bo = te[:, i, BASE + 1 : BASE + 1 + W]
                            bo = bo.rearrange("p (c t) -> p c t", t=2)[:, :, 0]
                            so = t123[:, cb + 1 : cb + 1 + W]
                            so = so.rearrange("p (c t) -> p c t", t=2)[:, :, 0]
                            slots.append((bo, so))
                for g in range(0, len(slots), 4):
                    grp = slots[g : g + 4]
                    psum = ps_pool.tile([128, 4 * MMW], F32, tag="ps")
                    for m, (brhs, srhs) in enumerate(grp):
                        nc.tensor.matmul(
                            out=psum[:, m * MMW : (m + 1) * MMW],
                            lhsT=wi[:],
                            rhs=brhs,
                            start=True,
                            stop=False,
                        )
                    for m, (brhs, srhs) in enumerate(grp):
                        nc.tensor.matmul(
                            out=psum[:, m * MMW : (m + 1) * MMW],
                            lhsT=wni[:],
                            rhs=srhs,
                            start=False,
                            stop=True,
                        )
                    col = 1 + r * NGRP + g // 4
                    nc.scalar.activation(
                        out=psum[:, 0 : len(grp) * MMW],
                        in_=psum[:, 0 : len(grp) * MMW],
                        func=mybir.ActivationFunctionType.Abs,
                        accum_out=stage[:, col : col + 1],
                    )

                # --- DVE chain: 2 k=0 ops + 3 j-merged trios + (one
                # final read). All ops write the shared scratch tile so
                # WAW deps keep the chain contiguous under the scheduler.
                for n, (var, delta) in enumerate(K0_SCHED):
                    kind = "seed" if (n == 0 and r == 0) else "cont"
                    c0 = BASE + delta - 1
                    nc.vector._custom_dve(
                        ops[(kind, var)],
                        out=scratch[:, 0 : IMGS * W],
                        in0=base,
                        in1=to0[:, :, c0 : c0 + W],
                    ).ins.perf_max = 1
                in0b = te[:].rearrange("p a b -> p (a b)")
                in0b = in0b.rearrange("p (x c) -> p x c", x=1)
                in0b = in0b.broadcast_to((128, 3, ROW))
                for var, delta in TRIO_SCHED:
                    m0 = delta + 3
                    in1b = t123[:, m0 : m0 + 3 * ROW].rearrange(
                        "p (j c) -> p j c", j=3
                    )
                    nc.vector._custom_dve(
                        ops[("cont", var)],
                        out=scratch[:],
                        in0=in0b,
                        in1=in1b,
                    ).ins.perf_max = 1
            nc.vector._custom_dve(
                ops["read"],
                out=stage[:, 0:1],
                in0=scratch[:, 0:1],
            )
            nc.sync.dma_start(out=OUT[:], in_=stage[:])
    return nc


_NC = None


def _get_nc():
    global _NC
    if _NC is None:
        _NC = _build_nc()
        if not _NC.is_finalized():
            _NC.finalize()
    return _NC


def _prep_shards(x: np.ndarray) -> list[dict[str, np.ndarray]]:
    """bf16-cast, circular pad, blockify into the flat per-core layout."""
    imgs = np.ascontiguousarray(x.reshape(B * C, H, W), dtype=np.float32)

    def to_bf16(a32):
        b = a32.view(np.uint32)
        return ((b + 0x7FFF + ((b >> 16) & 1)) >> 16).astype(np.uint16)

    imgs_b = to_bf16(imgs)  # (24, H, W) uint16 view of bf16
    HPAD = H + 3
    even = np.zeros((B * C, HPAD, WP), dtype=np.uint16)
    even[:, :H, BASE : BASE + W] = imgs_b
    even[:, :H, :BASE] = imgs_b[:, :, W - BASE :]
    even[:, :H, BASE + W : BASE + W + 3] = imgs_b[:, :, :3]
    even[:, H:, :] = even[:, :3, :]

    I = np.eye(128, dtype=np.float32)
    wi = to_bf16(I)
    wni = to_bf16(-I)

    shards = even.reshape(NCORES, IMGS, HPAD, WP)
    out = []
    pcorr = []
    for n in range(NCORES):
        t = shards[n].transpose(1, 0, 2)  # (HPAD, IMGS, WP)
        blk = np.empty((NBLK, ROWS_BLK, IMGS, WP), dtype=np.uint16)
        for r in range(NBLK):
            blk[r] = t[r * RB : r * RB + ROWS_BLK]
        flat = np.concatenate([blk.reshape(-1), np.zeros(8, np.uint16)])
        out.append({"x": flat, "wi": wi, "wni": wni})
        # Exact correction for the merged-trio ops' pad-column terms:
        # streams span full WP rows, so a-columns [0,BASE) u [BASE+W,WP)
        # contribute deterministic extra |a-b| terms (b read at flat
        # offset +j*ROW+delta, exactly as the device tile is laid out).
        af = (flat.astype(np.uint32) << 16).view(np.float32).astype(np.float64)
        rowbase = (
            (ROWS_BLK * np.arange(NBLK)[:, None] + np.arange(128)[None, :])
            * ROW
        )
        P = 0.0
        for j in (1, 2, 3):
            for var, dlt in TRIO_SCHED:
                for i in range(IMGS):
                    for c in (0, 2, BASE + W, BASE + W + 2):
                        ai = rowbase + i * WP + c
                        bi = ai + j * ROW + dlt
                        alo, ahi = af[ai], af[ai + 1]
                        blo, bhi = af[bi], af[bi + 1]
                        t = np.abs(alo - blo) + np.abs(ahi - bhi)
                        if var != "nd3":
                            t += np.abs(alo - bhi)
                        if var != "nd4":
                            t += np.abs(ahi - blo)
                        P += t.sum()
        pcorr.append(P)
    return out, pcorr


def _run(x: np.ndarray, trace: bool = False):
    import ml_dtypes

    nc = _get_nc()
    in_maps, pcorr = _prep_shards(x)
    in_maps = [
        {k: v.view(ml_dtypes.bfloat16) for k, v in m.items()} for m in in_maps
    ]
    res = run_bass_kernel_spmd(
        nc, in_maps, core_ids=list(range(NCORES)), trace=trace
    )
    total = 0.0
    for r, pc in zip(res.results, pcorr):
        total += r["out"].astype(np.float64).sum() - pc
    val = WEIGHT * 2.0 * total / float(B * C * H * W)
    return np.float32(val), res


def kernel(x: np.ndarray) -> np.ndarray:
    x = np.asarray(x, dtype=np.float32)
    val, _ = _run(x, trace=False)
    return val
